# revision 16
# baseline (speedup 1.0000x reference)
"""Trainium2 8-core attention kernel v9 (N=8192, D=512, Q==K shared projection).

Projection-free formulation; BOTH big GEMM families in fp8e4 DoubleRow.

Scores:  scores^T[j, i] = e_j . (G e_i + h),  G = W_qk^T W_qk, h = W_qk^T b_qk
computed as T = W_qk E^T_loc + b 1^T (bias via Identity-activation evac), then
etlg = W_qk^T T = G E^T_loc + h 1^T; the score contraction runs fp8 DoubleRow
(lhsT = raw E^T fp8, rhs = etlg fp8).

Value side:  attn @ V = (P @ E) W_v^T + b_v.  The softmax DIAGONAL
(p_ii ~ e^14, which overflows fp8) is split out exactly: its logit is
extracted from PSUM (masked accum), then suppressed (-2^17) before the Exp
writes P directly in fp8e4; off-diagonal P has logits ~N(0,0.8) and fits
fp8's normal range.  (P@E)^T then runs fp8 DoubleRow over column-chunk
pairs; the diagonal contribution p_ii * (E_loc W_v^T)_i is added back from a
small bf16 local V projection before normalization.

The diagonal lands at compile-time-fixed loop positions on every core by
ROTATING each core's column-chunk order: core c processes global chunk
(q + 8c) % 64 at loop position q, so diag chunks are always q = rb*4 + jj.
The et8/en8 host buffers are built in that per-core order (PV/l-sums are
order-invariant).

All DRAM inputs are host-prepared in the exact SBUF layout; input streams
split over HWDGE (sync) and SWDGE (gpsimd) queues; l-sum accumulation
alternates DVE / GpSimd.  10 junk matmuls warm the PE clock while the first
DMAs land.
"""

import ml_dtypes
import numpy as np

import concourse.bass as bass
import concourse.mybir as mybir
import concourse.tile as tile
from concourse import bacc
from concourse.bass_utils import run_bass_kernel_spmd

N = 8192          # sequence length
F = 512           # input features
D = 512           # output features (head dim)
CORES = 8
NL = N // CORES   # local rows per core (1024)
SCALE = 1.0 / float(np.sqrt(D))
BIG = 131072.0    # 2**17, diagonal suppression constant

FC = F // 128     # 4 f-chunks
DC = D // 128     # 4 d-chunks
RB = NL // 512    # 2 row-blocks of 512
CC = N // 128     # 64 column chunks
SW = 512          # slab width over N for streamed embedding tensors
NSLAB = N // SW   # 16

f32 = mybir.dt.float32
bf16 = mybir.dt.bfloat16
f8 = mybir.dt.float8e4
DR = mybir.MatmulPerfMode.DoubleRow
ACT = mybir.ActivationFunctionType
ALU = mybir.AluOpType

_NC = None
LAST_RESULT = None


def build_kernel():
    nc = bacc.Bacc(target_bir_lowering=False)

    # all in exact SBUF layout, host-prepared (et8/en8 per-core chunk-rotated)
    et8d = nc.declare_dram_parameter("et8", [128, NSLAB * FC * SW], f8, isOutput=False)
    en8d = nc.declare_dram_parameter("en8", [128, CC * F], f8, isOutput=False)
    etld = nc.declare_dram_parameter("etl", [128, RB * FC * SW], bf16, isOutput=False)
    wnTd = nc.declare_dram_parameter("wnT", [128, FC * D], bf16, isOutput=False)
    wnd = nc.declare_dram_parameter("wn", [128, DC * F], bf16, isOutput=False)
    wvd = nc.declare_dram_parameter("wv", [128, FC * D], bf16, isOutput=False)
    idmd = nc.declare_dram_parameter("idm", [128, 128], f32, isOutput=False)
    bqk = nc.declare_dram_parameter("bqk", [D], f32, isOutput=False)
    bv = nc.declare_dram_parameter("bv", [D], f32, isOutput=False)
    out = nc.declare_dram_parameter("out", [NL, D], f32, isOutput=True)

    with tile.TileContext(nc) as tc:
        with (
            tc.tile_pool(name="persist", bufs=1) as persist,
            tc.tile_pool(name="work", bufs=2) as work,
            tc.tile_pool(name="ps", bufs=3, space="PSUM") as ps,
        ):
            # ---- HAM warmup: junk matmuls keep PE busy while DMAs land ----
            junk = persist.tile([128, 512], bf16)
            nc.vector.memset(junk, 0.25)
            junk_ps = ps.tile([128, 512], f32, tag="mm_ps")
            for _ in range(12):
                nc.tensor.matmul(junk_ps, junk[:, :128], junk,
                                 start=True, stop=True, skip_group_check=True)

            # ---- DMA issue order: gating tensors first on HWDGE ----
            wnT = persist.tile([128, FC * D], bf16)    # W_qk^T, f-chunk fc at cols fc*D
            wn = persist.tile([128, DC * F], bf16)     # W_qk,   d-chunk dc at cols dc*F
            wv = persist.tile([128, FC * D], bf16)     # W_v^T,  f-chunk fc at cols fc*D
            # E^T local, nb-major: (nb, fc) block at cols nb*FC*512 + fc*512
            etl = persist.tile([128, RB * FC * SW], bf16)
            idm = persist.tile([128, 128], f32)        # BIG * identity
            nc.sync.dma_start(out=wnT[:, :], in_=wnTd[:, :])
            nc.sync.dma_start(out=etl[:, :2048], in_=etld[:, :2048])

            bqk_d = persist.tile([128, DC], f32)
            nc.gpsimd.dma_start(out=bqk_d, in_=bqk.rearrange("(c p) -> p c", p=128))
            bv_bc = persist.tile([128, D], f32)
            bv_ap = bv[:]
            nc.gpsimd.dma_start(out=bv_bc, in_=bass.AP(
                tensor=bv_ap.tensor, offset=bv_ap.offset,
                ap=[[0, 128], *bv_ap.ap]))
            nc.gpsimd.dma_start(out=wv[:, :], in_=wvd[:, :])

            # et8: slab-major [p, (s, fc, t)] on HWDGE; en8 on SWDGE
            et8 = persist.tile([128, NSLAB * FC * SW], f8)
            en8 = persist.tile([128, CC * F], f8)

            def et8_slab(sl):
                nc.sync.dma_start(
                    out=et8[:, sl * FC * SW:(sl + 1) * FC * SW],
                    in_=et8d[:, sl * FC * SW:(sl + 1) * FC * SW])

            nc.sync.dma_start(out=wn[:, :], in_=wnd[:, :])
            et8_slab(0)
            et8_slab(1)
            nc.sync.dma_start(out=etl[:, 2048:], in_=etld[:, 2048:])
            nc.sync.dma_start(out=idm, in_=idmd[:, :])
            for sl in range(2, NSLAB):
                et8_slab(sl)
            for sl in range(NSLAB):
                nc.gpsimd.dma_start(
                    out=en8[:, sl * FC * SW:(sl + 1) * FC * SW],
                    in_=en8d[:, sl * FC * SW:(sl + 1) * FC * SW])

            ones_b = persist.tile([128, 1], bf16)
            nc.vector.memset(ones_b, 1.0)

            # ---- prep: T = W_qk E^T_loc + b 1^T ; etlg8 = W_qk^T T (fp8) ----
            T_sb = persist.tile([128, DC * NL], bf16)   # d-chunk dc at cols dc*NL
            etlg8 = persist.tile([128, FC * NL], f8)    # f-chunk fc at cols fc*NL
            V_nb = persist.tile([128, 8 * D], bf16)     # local V (no bias), ic at ic*D

            def emit_T(nb):
                r0 = nb * 512
                for dc in range(DC):
                    t_ps = ps.tile([128, 512], f32, tag="mm_ps")
                    for fc in range(FC):
                        nc.tensor.matmul(
                            t_ps,
                            wnT[:, fc * D + dc * 128: fc * D + (dc + 1) * 128],
                            etl[:, nb * FC * SW + fc * SW: nb * FC * SW + (fc + 1) * SW],
                            start=(fc == 0), stop=(fc == FC - 1),
                        )
                    nc.vector.tensor_scalar_add(
                        out=T_sb[:, dc * NL + r0: dc * NL + r0 + 512], in0=t_ps,
                        scalar1=bqk_d[:, dc:dc + 1])

            def emit_etlg(nb):
                r0 = nb * 512
                for fp in range(FC):
                    g_ps = ps.tile([128, 512], f32, tag="mm_ps")
                    for dc in range(DC):
                        nc.tensor.matmul(
                            g_ps,
                            wn[:, dc * F + fp * 128: dc * F + (fp + 1) * 128],
                            T_sb[:, dc * NL + r0: dc * NL + r0 + 512],
                            start=(dc == 0), stop=(dc == DC - 1),
                        )
                    nc.vector.tensor_copy(
                        out=etlg8[:, fp * NL + r0: fp * NL + r0 + 512], in_=g_ps)

            def emit_Vnb():
                for ic in range(8):
                    v_ps = ps.tile([128, 512], f32, tag="mm_ps")
                    for fc in range(FC):
                        nc.tensor.matmul(
                            v_ps,
                            etl[:, (ic // 4) * FC * SW + fc * SW + (ic % 4) * 128:
                                (ic // 4) * FC * SW + fc * SW + (ic % 4) * 128 + 128],
                            wv[:, fc * D:(fc + 1) * D],
                            start=(fc == 0), stop=(fc == FC - 1),
                        )
                    nc.vector.tensor_copy(out=V_nb[:, ic * D:(ic + 1) * D], in_=v_ps)

            emit_T(0)
            emit_etlg(0)

            # ---- attention: 2 row-blocks of 512 local rows ----
            for rb in range(RB):
                r0 = rb * 512
                pvt_ps = [
                    ps.tile([128, 512], f32, tag="pvt_ps", bufs=4, name=f"pvt{rb}_{fb}")
                    for fb in range(FC)
                ]
                lacc = [work.tile([128, 512], f32, tag="lacc", bufs=4,
                                  name=f"lacc{rb}_{h}") for h in range(2)]
                ppl = work.tile([128, 4], f32, tag="ppl", bufs=2)     # diag logits
                pp4 = work.tile([128, 4], f32, tag="pp4", bufs=2)     # exp(diag)
                dg = [work.tile([128, 128], bf16, tag="dg", bufs=8,
                                name=f"dg_{rb}_{j}") for j in range(4)]
                for u in range(CC // 2):
                    if rb == 0 and u == 2:
                        emit_T(1)
                        emit_etlg(1)
                        emit_Vnb()
                    p8 = work.tile([128, 2 * 512], f8, tag="p8", bufs=4)
                    for h in range(2):
                        cc = 2 * u + h
                        sl, t = divmod(cc, FC)
                        st_ps = ps.tile([128, 512], f32, tag="mm_ps")
                        for g in range(2):
                            lhsT = et8[:, sl * FC * SW + 2 * g * SW:
                                       sl * FC * SW + (2 * g + 2) * SW].rearrange(
                                "p (k n) -> p k n", k=2)[:, :, t * 128:(t + 1) * 128]
                            rhs = etlg8[:, 2 * g * NL:(2 * g + 2) * NL].rearrange(
                                "p (k n) -> p k n", k=2)[:, :, r0:r0 + 512]
                            nc.tensor.matmul(
                                st_ps, lhsT, rhs,
                                start=(g == 0), stop=(g == 1), perf_mode=DR,
                            )
                        jj = cc - (28 + rb * 4)
                        if 0 <= jj < 4:
                            # extract diag logit (masked accum), then suppress
                            sli = st_ps[:, jj * 128:(jj + 1) * 128]
                            trash = work.tile([128, 128], f32, tag="trash", bufs=2)
                            nc.vector.scalar_tensor_tensor(
                                out=trash, in0=sli, scalar=1.0 / BIG, in1=idm,
                                op0=ALU.mult, op1=ALU.mult,
                                accum_out=ppl[:, jj:jj + 1],
                            )
                            nc.vector.scalar_tensor_tensor(
                                out=sli, in0=sli, scalar=1.0, in1=idm,
                                op0=ALU.mult, op1=ALU.subtract,
                            )
                        nc.scalar.activation(
                            out=p8[:, h * 512:(h + 1) * 512], in_=st_ps,
                            func=ACT.Exp, scale=SCALE,
                        )
                        eng = nc.gpsimd if h == 0 else nc.vector
                        if u == 0:
                            eng.tensor_copy(out=lacc[h], in_=p8[:, h * 512:(h + 1) * 512])
                        else:
                            eng.tensor_add(lacc[h], lacc[h], p8[:, h * 512:(h + 1) * 512])
                        if cc == 31 + rb * 4:
                            nc.scalar.activation(out=pp4, in_=ppl,
                                                 func=ACT.Exp, scale=SCALE)
                    p8r = p8[:, :].rearrange("p (k n) -> p k n", k=2)
                    for fb in range(FC):
                        lhsT = en8[:, 2 * u * F:(2 * u + 2) * F].rearrange(
                            "p (k n) -> p k n", k=2)[:, :, fb * 128:(fb + 1) * 128]
                        nc.tensor.matmul(
                            pvt_ps[fb], lhsT, p8r,
                            start=(u == 0), stop=(u == CC // 2 - 1), perf_mode=DR,
                        )
                    if u == 17 + rb:
                        s2 = work.tile([128, 4], f32, tag="s2", bufs=2)
                        nc.vector.tensor_scalar_mul(out=s2, in0=pp4, scalar1=1.0 / BIG)
                        for j in range(4):
                            nc.vector.tensor_scalar_mul(
                                out=dg[j], in0=idm, scalar1=s2[:, j:j + 1])

                # rb epilogue.  l-sums first (8 tiny matmuls, one per
                # accumulator half -- no merge, so the PE waits only on the
                # last chain tail), then (P@E)^T evacs + W_v projection for
                # all j (keeps PE busy while linv/t2 compute on DVE), then
                # normalize+diag-add-back STTs (alternating DVE/GpSimd) + DMA.
                laccb = [work.tile([128, 512], bf16, tag="laccb", bufs=4,
                                   name=f"laccb{rb}_{h}") for h in range(2)]
                nc.scalar.activation(out=laccb[0], in_=lacc[0], func=ACT.Copy)
                nc.vector.tensor_copy(out=laccb[1], in_=lacc[1])
                l_ps = ps.tile([128, 8], f32, tag="l_ps", bufs=1)
                for half in range(2):
                    for j in range(4):
                        nc.tensor.matmul(
                            l_ps[:, half * 4 + j:half * 4 + j + 1],
                            laccb[half][:, j * 128:(j + 1) * 128],
                            ones_b,
                            start=True, stop=True, skip_group_check=True,
                        )
                lsum = work.tile([128, 4], f32, tag="lsum")
                nc.vector.tensor_add(lsum, l_ps[:, 0:4], pp4)
                nc.vector.tensor_add(lsum, lsum, l_ps[:, 4:8])
                linv = work.tile([128, 4], f32, tag="linv")
                nc.vector.reciprocal(out=linv, in_=lsum)
                o_pss = []
                for j in range(4):
                    ptb = [
                        work.tile([128, 128], bf16, tag="ptb", bufs=8,
                                  name=f"ptb{rb}_{j}_{fb}")
                        for fb in range(FC)
                    ]
                    for fb in range(FC):
                        if fb % 2 == 0:
                            nc.vector.tensor_copy(
                                out=ptb[fb], in_=pvt_ps[fb][:, j * 128:(j + 1) * 128])
                        else:
                            nc.scalar.activation(
                                out=ptb[fb], in_=pvt_ps[fb][:, j * 128:(j + 1) * 128],
                                func=ACT.Copy)
                    o_ps = ps.tile([128, D], f32, tag="mm_ps")
                    for fb in range(FC):
                        nc.tensor.matmul(
                            o_ps,
                            ptb[fb],
                            wv[:, fb * D:(fb + 1) * D],
                            start=(fb == 0), stop=False,
                        )
                    ic = rb * 4 + j
                    nc.tensor.matmul(
                        o_ps, dg[j], V_nb[:, ic * D:(ic + 1) * D],
                        start=False, stop=True,
                    )
                    o_pss.append(o_ps)

                for j in range(4):
                    o_t = work.tile([128, D], f32, tag="o_t", bufs=4)
                    nc.vector.scalar_tensor_tensor(
                        out=o_t, in0=o_pss[j], scalar=linv[:, j:j + 1],
                        in1=bv_bc, op0=ALU.mult,
                        op1=ALU.add,
                    )
                    deng = nc.sync if (rb == 1 or j % 2 == 0) else nc.gpsimd
                    deng.dma_start(
                        out=out[r0 + j * 128: r0 + (j + 1) * 128, :], in_=o_t)

    nc.compile()
    return nc


def _get_nc():
    global _NC
    if _NC is None:
        _NC = build_kernel()
    return _NC


def kernel(embedding, W_qk, b_qk, W_v, b_v):
    global LAST_RESULT
    E = np.ascontiguousarray(np.asarray(embedding, dtype=np.float32))  # [N, F]
    E8 = E.astype(ml_dtypes.float8_e4m3fn)
    chunks = E8.reshape(CC, 128, F)            # (G, p, f) global column chunks

    def prep_w(M):
        M = np.ascontiguousarray(np.asarray(M, dtype=np.float32)).astype(ml_dtypes.bfloat16)
        return np.ascontiguousarray(
            M.reshape(4, 128, M.shape[1]).transpose(1, 0, 2).reshape(128, 4 * M.shape[1]))

    wqk_f = np.ascontiguousarray(np.asarray(W_qk, dtype=np.float32))
    wnT = prep_w(wqk_f.T)
    wn = prep_w(wqk_f)
    wv = prep_w(np.asarray(W_v, dtype=np.float32).T)
    bqk = np.ascontiguousarray(np.asarray(b_qk, dtype=np.float32))
    bv = np.ascontiguousarray(np.asarray(b_v, dtype=np.float32))
    idm = np.ascontiguousarray(BIG * np.eye(128, dtype=np.float32))

    Eb = E.astype(ml_dtypes.bfloat16)
    in_maps = []
    for c in range(CORES):
        order = (np.arange(CC) + 8 * c + 36) % CC
        rot = chunks[order]                     # (q, p, f)
        # et8: [p, (s, fc, q%4, tt)] = E[G(q)*128 + tt, fc*128 + p]
        et8 = np.ascontiguousarray(
            rot.reshape(NSLAB, 4, 128, FC, 128)  # (s, qm, tt, fc, p)
            .transpose(4, 0, 3, 1, 2)
            .reshape(128, NSLAB * FC * SW))
        # en8: [p, (q, f)] = E[G(q)*128 + p, f]
        en8 = np.ascontiguousarray(
            rot.transpose(1, 0, 2).reshape(128, CC * F))
        # etl: [128, (nb, fc, r)] = E[c*NL + nb*512 + r, fc*128+p] in bf16
        etl = np.ascontiguousarray(
            Eb[c * NL:(c + 1) * NL]
            .reshape(RB, SW, FC, 128)
            .transpose(3, 0, 2, 1)
            .reshape(128, RB * FC * SW))
        in_maps.append({
            "et8": et8, "en8": en8, "etl": etl,
            "wnT": wnT, "wn": wn, "wv": wv, "idm": idm,
            "bqk": bqk, "bv": bv,
        })

    nc = _get_nc()
    res = run_bass_kernel_spmd(nc, in_maps, core_ids=list(range(CORES)))
    LAST_RESULT = res
    return np.concatenate(
        [np.asarray(res.results[i]["out"]) for i in range(CORES)], axis=0
    )


# revision 17
# speedup vs baseline: 1.1693x; 1.1693x over previous
"""Trainium2 8-core attention kernel v9 (N=8192, D=512, Q==K shared projection).

Projection-free formulation; BOTH big GEMM families in fp8e4 DoubleRow.

Scores:  scores^T[j, i] = e_j . (G e_i + h),  G = W_qk^T W_qk, h = W_qk^T b_qk
computed as T = W_qk E^T_loc + b 1^T (bias via Identity-activation evac), then
etlg = W_qk^T T = G E^T_loc + h 1^T; the score contraction runs fp8 DoubleRow
(lhsT = raw E^T fp8, rhs = etlg fp8).

Value side:  attn @ V = (P @ E) W_v^T + b_v.  The softmax DIAGONAL
(p_ii ~ e^14, which overflows fp8) is split out exactly: its logit is
extracted from PSUM (masked accum), then suppressed (-2^17) before the Exp
writes P directly in fp8e4; off-diagonal P has logits ~N(0,0.8) and fits
fp8's normal range.  (P@E)^T then runs fp8 DoubleRow over column-chunk
pairs; the diagonal contribution p_ii * (E_loc W_v^T)_i is added back from a
small bf16 local V projection before normalization.

The diagonal lands at compile-time-fixed loop positions on every core by
ROTATING each core's column-chunk order: core c processes global chunk
(q + 8c) % 64 at loop position q, so diag chunks are always q = rb*4 + jj.
The et8/en8 host buffers are built in that per-core order (PV/l-sums are
order-invariant).

All DRAM inputs are host-prepared in the exact SBUF layout; input streams
split over HWDGE (sync) and SWDGE (gpsimd) queues; l-sum accumulation
alternates DVE / GpSimd.  10 junk matmuls warm the PE clock while the first
DMAs land.
"""

import ml_dtypes
import numpy as np

import concourse.bass as bass
import concourse.mybir as mybir
import concourse.tile as tile
from concourse import bacc
from concourse.bass_utils import run_bass_kernel_spmd

N = 8192          # sequence length
F = 512           # input features
D = 512           # output features (head dim)
CORES = 8
NL = N // CORES   # local rows per core (1024)
SCALE = 1.0 / float(np.sqrt(D))
BIG = 131072.0    # 2**17, diagonal suppression constant

FC = F // 128     # 4 f-chunks
DC = D // 128     # 4 d-chunks
RB = NL // 512    # 2 row-blocks of 512
CC = N // 128     # 64 column chunks
SW = 512          # slab width over N for streamed embedding tensors
NSLAB = N // SW   # 16

f32 = mybir.dt.float32
bf16 = mybir.dt.bfloat16
f8 = mybir.dt.float8e4
DR = mybir.MatmulPerfMode.DoubleRow
ACT = mybir.ActivationFunctionType
ALU = mybir.AluOpType

_NC = None
LAST_RESULT = None


def build_kernel():
    nc = bacc.Bacc(target_bir_lowering=False)

    # all in exact SBUF layout, host-prepared (et8/en8 per-core chunk-rotated)
    et8d = nc.declare_dram_parameter("et8", [128, NSLAB * FC * SW], f8, isOutput=False)
    en8d = nc.declare_dram_parameter("en8", [128, CC * F], f8, isOutput=False)
    etld = nc.declare_dram_parameter("etl", [128, RB * FC * SW], bf16, isOutput=False)
    wnTd = nc.declare_dram_parameter("wnT", [128, FC * D], bf16, isOutput=False)
    wnd = nc.declare_dram_parameter("wn", [128, DC * F], bf16, isOutput=False)
    wvd = nc.declare_dram_parameter("wv", [128, FC * D], bf16, isOutput=False)
    idmd = nc.declare_dram_parameter("idm", [128, 128], f32, isOutput=False)
    bqk = nc.declare_dram_parameter("bqk", [D], f32, isOutput=False)
    bv = nc.declare_dram_parameter("bv", [D], f32, isOutput=False)
    out = nc.declare_dram_parameter("out", [NL, D], f32, isOutput=True)

    with tile.TileContext(nc) as tc:
        with (
            tc.tile_pool(name="persist", bufs=1) as persist,
            tc.tile_pool(name="work", bufs=2) as work,
            tc.tile_pool(name="ps", bufs=3, space="PSUM") as ps,
        ):
            # ---- HAM warmup: junk matmuls keep PE busy while DMAs land ----
            junk = persist.tile([128, 512], bf16)
            nc.vector.memset(junk, 0.25)
            junk_ps = ps.tile([128, 512], f32, tag="mm_ps")
            for _ in range(12):
                nc.tensor.matmul(junk_ps, junk[:, :128], junk,
                                 start=True, stop=True, skip_group_check=True)

            # ---- DMA issue order: gating tensors first on HWDGE ----
            wnT = persist.tile([128, FC * D], bf16)    # W_qk^T, f-chunk fc at cols fc*D
            wn = persist.tile([128, DC * F], bf16)     # W_qk,   d-chunk dc at cols dc*F
            wv = persist.tile([128, FC * D], bf16)     # W_v^T,  f-chunk fc at cols fc*D
            # E^T local, nb-major: (nb, fc) block at cols nb*FC*512 + fc*512
            etl = persist.tile([128, RB * FC * SW], bf16)
            idm = persist.tile([128, 128], f32)        # BIG * identity
            nc.sync.dma_start(out=wnT[:, :], in_=wnTd[:, :])
            nc.sync.dma_start(out=etl[:, :2048], in_=etld[:, :2048])

            bqk_d = persist.tile([128, DC], f32)
            nc.gpsimd.dma_start(out=bqk_d, in_=bqk.rearrange("(c p) -> p c", p=128))
            bv_bc = persist.tile([128, D], f32)
            bv_ap = bv[:]
            nc.gpsimd.dma_start(out=bv_bc, in_=bass.AP(
                tensor=bv_ap.tensor, offset=bv_ap.offset,
                ap=[[0, 128], *bv_ap.ap]))
            nc.gpsimd.dma_start(out=wv[:, :], in_=wvd[:, :])

            # et8: slab-major [p, (s, fc, t)] on HWDGE; en8 on SWDGE
            et8 = persist.tile([128, NSLAB * FC * SW], f8)
            en8 = persist.tile([128, CC * F], f8)

            def et8_slab(sl):
                nc.sync.dma_start(
                    out=et8[:, sl * FC * SW:(sl + 1) * FC * SW],
                    in_=et8d[:, sl * FC * SW:(sl + 1) * FC * SW])

            nc.sync.dma_start(out=wn[:, :], in_=wnd[:, :])
            et8_slab(0)
            et8_slab(1)
            nc.sync.dma_start(out=etl[:, 2048:], in_=etld[:, 2048:])
            nc.sync.dma_start(out=idm, in_=idmd[:, :])
            for sl in range(2, NSLAB):
                et8_slab(sl)
            for sl in range(NSLAB):
                nc.gpsimd.dma_start(
                    out=en8[:, sl * FC * SW:(sl + 1) * FC * SW],
                    in_=en8d[:, sl * FC * SW:(sl + 1) * FC * SW])

            ones_b = persist.tile([128, 1], bf16)
            nc.vector.memset(ones_b, 1.0)

            # ---- prep: T = W_qk E^T_loc + b 1^T ; etlg8 = W_qk^T T (fp8) ----
            T_sb = persist.tile([128, DC * NL], bf16)   # d-chunk dc at cols dc*NL
            etlg8 = persist.tile([128, FC * NL], f8)    # f-chunk fc at cols fc*NL
            V_nb = persist.tile([128, 8 * D], bf16)     # local V (no bias), ic at ic*D

            def emit_T(nb):
                r0 = nb * 512
                for dc in range(DC):
                    t_ps = ps.tile([128, 512], f32, tag="mm_ps")
                    for fc in range(FC):
                        nc.tensor.matmul(
                            t_ps,
                            wnT[:, fc * D + dc * 128: fc * D + (dc + 1) * 128],
                            etl[:, nb * FC * SW + fc * SW: nb * FC * SW + (fc + 1) * SW],
                            start=(fc == 0), stop=(fc == FC - 1),
                        )
                    nc.vector.tensor_scalar_add(
                        out=T_sb[:, dc * NL + r0: dc * NL + r0 + 512], in0=t_ps,
                        scalar1=bqk_d[:, dc:dc + 1])

            def emit_etlg(nb):
                r0 = nb * 512
                for fp in range(FC):
                    g_ps = ps.tile([128, 512], f32, tag="mm_ps")
                    for dc in range(DC):
                        nc.tensor.matmul(
                            g_ps,
                            wn[:, dc * F + fp * 128: dc * F + (fp + 1) * 128],
                            T_sb[:, dc * NL + r0: dc * NL + r0 + 512],
                            start=(dc == 0), stop=(dc == DC - 1),
                        )
                    nc.vector.tensor_copy(
                        out=etlg8[:, fp * NL + r0: fp * NL + r0 + 512], in_=g_ps)

            def emit_Vnb():
                for ic in range(8):
                    v_ps = ps.tile([128, 512], f32, tag="mm_ps")
                    for fc in range(FC):
                        nc.tensor.matmul(
                            v_ps,
                            etl[:, (ic // 4) * FC * SW + fc * SW + (ic % 4) * 128:
                                (ic // 4) * FC * SW + fc * SW + (ic % 4) * 128 + 128],
                            wv[:, fc * D:(fc + 1) * D],
                            start=(fc == 0), stop=(fc == FC - 1),
                        )
                    nc.vector.tensor_copy(out=V_nb[:, ic * D:(ic + 1) * D], in_=v_ps)

            emit_T(0)
            emit_etlg(0)

            # ---- attention: 2 row-blocks of 512 local rows ----
            for rb in range(RB):
                r0 = rb * 512
                pvt_ps = [
                    ps.tile([128, 512], f32, tag="pvt_ps", bufs=4, name=f"pvt{rb}_{fb}")
                    for fb in range(FC)
                ]
                lacc = [work.tile([128, 512], f32, tag="lacc", bufs=4,
                                  name=f"lacc{rb}_{h}") for h in range(2)]
                ppl = work.tile([128, 4], f32, tag="ppl", bufs=2)     # diag logits
                pp4 = work.tile([128, 4], f32, tag="pp4", bufs=2)     # exp(diag)
                dg = [work.tile([128, 128], bf16, tag="dg", bufs=8,
                                name=f"dg_{rb}_{j}") for j in range(4)]
                for u in range(CC // 2):
                    if rb == 0 and u == 2:
                        emit_T(1)
                        emit_etlg(1)
                        emit_Vnb()
                    p8 = work.tile([128, 2 * 512], f8, tag="p8", bufs=4)
                    for h in range(2):
                        cc = 2 * u + h
                        sl, t = divmod(cc, FC)
                        st_ps = ps.tile([128, 512], f32, tag="mm_ps")
                        for g in range(2):
                            lhsT = et8[:, sl * FC * SW + 2 * g * SW:
                                       sl * FC * SW + (2 * g + 2) * SW].rearrange(
                                "p (k n) -> p k n", k=2)[:, :, t * 128:(t + 1) * 128]
                            rhs = etlg8[:, 2 * g * NL:(2 * g + 2) * NL].rearrange(
                                "p (k n) -> p k n", k=2)[:, :, r0:r0 + 512]
                            nc.tensor.matmul(
                                st_ps, lhsT, rhs,
                                start=(g == 0), stop=(g == 1), perf_mode=DR,
                            )
                        jj = cc - (28 + rb * 4)
                        if 0 <= jj < 4:
                            # extract diag logit (masked accum), then suppress
                            sli = st_ps[:, jj * 128:(jj + 1) * 128]
                            trash = work.tile([128, 128], f32, tag="trash", bufs=2)
                            nc.vector.scalar_tensor_tensor(
                                out=trash, in0=sli, scalar=1.0 / BIG, in1=idm,
                                op0=ALU.mult, op1=ALU.mult,
                                accum_out=ppl[:, jj:jj + 1],
                            )
                            nc.vector.scalar_tensor_tensor(
                                out=sli, in0=sli, scalar=1.0, in1=idm,
                                op0=ALU.mult, op1=ALU.subtract,
                            )
                        nc.scalar.activation(
                            out=p8[:, h * 512:(h + 1) * 512], in_=st_ps,
                            func=ACT.Exp, scale=SCALE,
                        )
                        eng = nc.gpsimd if h == 0 else nc.vector
                        if u == 0:
                            eng.tensor_copy(out=lacc[h], in_=p8[:, h * 512:(h + 1) * 512])
                        else:
                            eng.tensor_add(lacc[h], lacc[h], p8[:, h * 512:(h + 1) * 512])
                        if cc == 31 + rb * 4:
                            nc.scalar.activation(out=pp4, in_=ppl,
                                                 func=ACT.Exp, scale=SCALE)
                    p8r = p8[:, :].rearrange("p (k n) -> p k n", k=2)
                    for fb in range(FC):
                        lhsT = en8[:, 2 * u * F:(2 * u + 2) * F].rearrange(
                            "p (k n) -> p k n", k=2)[:, :, fb * 128:(fb + 1) * 128]
                        nc.tensor.matmul(
                            pvt_ps[fb], lhsT, p8r,
                            start=(u == 0), stop=(u == CC // 2 - 1), perf_mode=DR,
                        )
                    if u == 17 + rb:
                        s2 = work.tile([128, 4], f32, tag="s2", bufs=2)
                        nc.vector.tensor_scalar_mul(out=s2, in0=pp4, scalar1=1.0 / BIG)
                        for j in range(4):
                            nc.vector.tensor_scalar_mul(
                                out=dg[j], in0=idm, scalar1=s2[:, j:j + 1])

                # rb epilogue.  l-sums first (8 tiny matmuls, one per
                # accumulator half -- no merge, so the PE waits only on the
                # last chain tail), then (P@E)^T evacs + W_v projection for
                # all j (keeps PE busy while linv/t2 compute on DVE), then
                # normalize+diag-add-back STTs (alternating DVE/GpSimd) + DMA.
                laccb = [work.tile([128, 512], bf16, tag="laccb", bufs=4,
                                   name=f"laccb{rb}_{h}") for h in range(2)]
                nc.scalar.activation(out=laccb[0], in_=lacc[0], func=ACT.Copy)
                nc.vector.tensor_copy(out=laccb[1], in_=lacc[1])
                l_ps = ps.tile([128, 8], f32, tag="l_ps", bufs=1)
                for half in range(2):
                    for j in range(4):
                        nc.tensor.matmul(
                            l_ps[:, half * 4 + j:half * 4 + j + 1],
                            laccb[half][:, j * 128:(j + 1) * 128],
                            ones_b,
                            start=True, stop=True, skip_group_check=True,
                        )
                lsum = work.tile([128, 4], f32, tag="lsum")
                nc.vector.tensor_add(lsum, l_ps[:, 0:4], pp4)
                nc.vector.tensor_add(lsum, lsum, l_ps[:, 4:8])
                linv = work.tile([128, 4], f32, tag="linv")
                nc.vector.reciprocal(out=linv, in_=lsum)
                for j in range(4):
                    ptb = [
                        work.tile([128, 128], bf16, tag="ptb", bufs=8,
                                  name=f"ptb{rb}_{j}_{fb}")
                        for fb in range(FC)
                    ]
                    for fb in range(FC):
                        if fb % 2 == 0:
                            nc.vector.tensor_copy(
                                out=ptb[fb], in_=pvt_ps[fb][:, j * 128:(j + 1) * 128])
                        else:
                            nc.scalar.activation(
                                out=ptb[fb], in_=pvt_ps[fb][:, j * 128:(j + 1) * 128],
                                func=ACT.Copy)
                    o_ps = ps.tile([128, D], f32, tag="mm_ps")
                    for fb in range(FC):
                        nc.tensor.matmul(
                            o_ps,
                            ptb[fb],
                            wv[:, fb * D:(fb + 1) * D],
                            start=(fb == 0), stop=False,
                        )
                    ic = rb * 4 + j
                    nc.tensor.matmul(
                        o_ps, dg[j], V_nb[:, ic * D:(ic + 1) * D],
                        start=False, stop=True,
                    )
                    o_t = work.tile([128, D], f32, tag="o_t", bufs=4)
                    nc.vector.scalar_tensor_tensor(
                        out=o_t, in0=o_ps, scalar=linv[:, j:j + 1],
                        in1=bv_bc, op0=ALU.mult,
                        op1=ALU.add,
                    )
                    deng = nc.sync if (rb == 1 or j % 2 == 0) else nc.gpsimd
                    deng.dma_start(
                        out=out[r0 + j * 128: r0 + (j + 1) * 128, :], in_=o_t)

    nc.compile()
    return nc


def _get_nc():
    global _NC
    if _NC is None:
        _NC = build_kernel()
    return _NC


def kernel(embedding, W_qk, b_qk, W_v, b_v):
    global LAST_RESULT
    E = np.ascontiguousarray(np.asarray(embedding, dtype=np.float32))  # [N, F]
    E8 = E.astype(ml_dtypes.float8_e4m3fn)
    chunks = E8.reshape(CC, 128, F)            # (G, p, f) global column chunks

    def prep_w(M):
        M = np.ascontiguousarray(np.asarray(M, dtype=np.float32)).astype(ml_dtypes.bfloat16)
        return np.ascontiguousarray(
            M.reshape(4, 128, M.shape[1]).transpose(1, 0, 2).reshape(128, 4 * M.shape[1]))

    wqk_f = np.ascontiguousarray(np.asarray(W_qk, dtype=np.float32))
    wnT = prep_w(wqk_f.T)
    wn = prep_w(wqk_f)
    wv = prep_w(np.asarray(W_v, dtype=np.float32).T)
    bqk = np.ascontiguousarray(np.asarray(b_qk, dtype=np.float32))
    bv = np.ascontiguousarray(np.asarray(b_v, dtype=np.float32))
    idm = np.ascontiguousarray(BIG * np.eye(128, dtype=np.float32))

    Eb = E.astype(ml_dtypes.bfloat16)
    in_maps = []
    for c in range(CORES):
        order = (np.arange(CC) + 8 * c + 36) % CC
        rot = chunks[order]                     # (q, p, f)
        # et8: [p, (s, fc, q%4, tt)] = E[G(q)*128 + tt, fc*128 + p]
        et8 = np.ascontiguousarray(
            rot.reshape(NSLAB, 4, 128, FC, 128)  # (s, qm, tt, fc, p)
            .transpose(4, 0, 3, 1, 2)
            .reshape(128, NSLAB * FC * SW))
        # en8: [p, (q, f)] = E[G(q)*128 + p, f]
        en8 = np.ascontiguousarray(
            rot.transpose(1, 0, 2).reshape(128, CC * F))
        # etl: [128, (nb, fc, r)] = E[c*NL + nb*512 + r, fc*128+p] in bf16
        etl = np.ascontiguousarray(
            Eb[c * NL:(c + 1) * NL]
            .reshape(RB, SW, FC, 128)
            .transpose(3, 0, 2, 1)
            .reshape(128, RB * FC * SW))
        in_maps.append({
            "et8": et8, "en8": en8, "etl": etl,
            "wnT": wnT, "wn": wn, "wv": wv, "idm": idm,
            "bqk": bqk, "bv": bv,
        })

    nc = _get_nc()
    res = run_bass_kernel_spmd(nc, in_maps, core_ids=list(range(CORES)))
    LAST_RESULT = res
    return np.concatenate(
        [np.asarray(res.results[i]["out"]) for i in range(CORES)], axis=0
    )
